# revision 27
# baseline (speedup 1.0000x reference)
"""Bass/Trainium2 kernel for nn_BivariateSpectral: batched smallest-eigenvalue of
S_b = sym(A + B*diag(x_b) + C*diag(y_b)), b = 0..32767, each 64x64, 8 NeuronCores.

Algorithm (per core, data-parallel over batch):
  Phase 1 - batched Lanczos (K steps) on D_b = (M_b + M_b^T)/64 = S_b/32.
    Batched matvec as shared 64x64 matmuls:
      D v = Ah v + Bh (x*v) + Ch (y*v) + x*(Bh^T v) + y*(Ch^T v)
    Layout: dim on partitions (two batch-halves packed as partitions 0-63 /
    64-127 with block-diagonal stationaries), batch on the free dim.
    Engine split per step: GpSimd does the x*v / y*v front products and the
    beta_{j-1}*v_{j-1} tail product; the PE accumulates all five matvec
    terms plus the elementwise combine terms and the recurrence tail into
    PSUM via identity matmuls; the Scalar engine squares w~ (beta product),
    takes the sqrt, and copies the alpha broadcast out of PSUM; the DVE
    only does the five unavoidable products it alone can do fast.
    alpha / |beta| rows are staged into tall SBUF tiles via SBUF->SBUF DMA.
  Phase 2 - PE-transpose of the tall tiles, then Sturm bisection on the
    K x K tridiagonals (batch on partitions), division-free char-poly
    recurrence, 3 passes x 8 shifts, split across DVE and GpSimd.
"""

import functools
import numpy as np

BATCH, DIM = 32768, 64
NCORES = 8
SHARD = BATCH // NCORES      # 4096 batch elems per core
NFREE = SHARD // 2           # 2048 free columns (two partition-halves)
CHUNK = 1024                 # per-group width (2 psum banks)
NCH = NFREE // CHUNK         # 2 groups
K = 32                       # Lanczos steps
NB = K - 1
ROWS_A = 2 * K               # 66 rows in alpha staging (2j+h)
ROWS_B = 2 * NB              # 64 rows in |beta| staging
TG = NFREE // 128            # 16 transpose column-groups
NS = 8                       # bisection shifts per pass
PASSES = 3
TSPLIT = 11                  # TG groups 0:TSPLIT on DVE, TSPLIT: on GpSimd
C_OP = np.float32(1.0 / 64.0)   # A,B,C host prescale: D = (M+M^T)/64 = S/32
OUT_SCALE = 16.0                # lam_S = 32 * 0.5 * (lo+hi)


def _v0_vec():
    rng = np.random.default_rng(1234)
    v = rng.standard_normal(DIM).astype(np.float64)
    v /= np.sqrt((v * v).sum())
    return v.astype(np.float32)


def _bd(m):
    """128x128 block-diagonal duplication of a 64x64 matrix."""
    out = np.zeros((128, 128), np.float32)
    out[:64, :64] = m
    out[64:, 64:] = m
    return out


def _bcast_flat(ap, ns=NS):
    """[128, T, 2] AP -> [128, ns, T, 2] via 0-step shift dim."""
    import concourse.bass as bass
    dims = list(ap.ap)
    return bass.AP(tensor=ap.tensor, offset=ap.offset, ap=[dims[0], [0, ns]] + dims[1:])


@functools.lru_cache(maxsize=4)
def _program(idx: int):
    import concourse.bacc as bacc
    import concourse.bass as bass
    import concourse.mybir as mybir
    import concourse.tile as tile
    from concourse.masks import make_identity

    F32 = mybir.dt.float32
    F32R = mybir.dt.float32r
    I32 = mybir.dt.int32
    OP = mybir.AluOpType
    ACTF = mybir.ActivationFunctionType

    nc = bacc.Bacc("TRN2", target_bir_lowering=False, debug=False)

    xy_in = nc.dram_tensor("xy", [128, 2 * NFREE], F32, kind="ExternalInput").ap()
    lms_in = nc.dram_tensor("lms", [128, 128], F32, kind="ExternalInput").ap()
    lbf_in = nc.dram_tensor("lbf", [128, 128], F32, kind="ExternalInput").ap()
    lcf_in = nc.dram_tensor("lcf", [128, 128], F32, kind="ExternalInput").ap()
    lbt_in = nc.dram_tensor("lbt", [128, 128], F32, kind="ExternalInput").ap()
    lct_in = nc.dram_tensor("lct", [128, 128], F32, kind="ExternalInput").ap()
    obd_in = nc.dram_tensor("obd", [128, 128], F32, kind="ExternalInput").ap()
    v0_in = nc.dram_tensor("v0", [128, 1], F32, kind="ExternalInput").ap()
    lam_out = nc.dram_tensor("lam", [SHARD], F32, kind="ExternalOutput").ap()

    with tile.TileContext(nc) as tc:
        with tc.tile_pool(name="persist", bufs=1) as persist:
            ta_tall = persist.tile([ROWS_A, NFREE], F32)   # alpha rows (2j+h)
            bb_tall = persist.tile([ROWS_B, NFREE], F32)   # |beta| rows (2j+h)
            epst = persist.tile([128, 1], F32)
            nc.vector.memset(epst[:], 1e-12)

            # ---------------- Phase 1: Lanczos ----------------
            with (
                tc.tile_pool(name="singles", bufs=1) as singles,
                tc.tile_pool(name="vpool", bufs=3) as vpool,
                tc.tile_pool(name="wtp", bufs=2) as wtp,
                tc.tile_pool(name="wk1", bufs=1) as wk1,
                tc.tile_pool(name="wk2", bufs=2) as wk2,
                tc.tile_pool(name="fr1", bufs=1) as fr1,
                tc.tile_pool(name="fr2", bufs=2) as fr2,
                tc.tile_pool(name="pwp", bufs=2, space="PSUM") as pwp,
                tc.tile_pool(name="psh", bufs=4, space="PSUM") as pshp,
            ):
                xyt = singles.tile([128, 2 * NFREE], F32)
                nc.sync.dma_start(out=xyt[:], in_=xy_in)
                lms = singles.tile([128, 128], F32)
                lbf = singles.tile([128, 128], F32)
                lcf = singles.tile([128, 128], F32)
                lbt = singles.tile([128, 128], F32)
                lct = singles.tile([128, 128], F32)
                obd = singles.tile([128, 128], F32)
                nc.sync.dma_start(out=lms[:], in_=lms_in)
                nc.sync.dma_start(out=lbf[:], in_=lbf_in)
                nc.sync.dma_start(out=lcf[:], in_=lcf_in)
                nc.sync.dma_start(out=lbt[:], in_=lbt_in)
                nc.sync.dma_start(out=lct[:], in_=lct_in)
                nc.sync.dma_start(out=obd[:], in_=obd_in)
                lms_r = singles.tile([128, 128], F32R)
                lbf_r = singles.tile([128, 128], F32R)
                lcf_r = singles.tile([128, 128], F32R)
                lbt_r = singles.tile([128, 128], F32R)
                lct_r = singles.tile([128, 128], F32R)
                obd_r = singles.tile([128, 128], F32R)
                nc.vector.tensor_copy(lms_r[:], lms[:])
                nc.vector.tensor_copy(lbf_r[:], lbf[:])
                nc.vector.tensor_copy(lcf_r[:], lcf[:])
                nc.vector.tensor_copy(lbt_r[:], lbt[:])
                nc.vector.tensor_copy(lct_r[:], lct[:])
                nc.vector.tensor_copy(obd_r[:], obd[:])
                identP = singles.tile([128, 128], F32)
                make_identity(nc, identP[:])
                identR = singles.tile([128, 128], F32R)
                nc.vector.tensor_copy(identR[:], identP[:])
                identN = singles.tile([128, 128], F32R)
                nc.vector.tensor_scalar(out=identN[:], in0=identP[:],
                                        scalar1=-1.0, scalar2=None, op0=OP.mult)
                v0t = singles.tile([128, 1], F32)
                nc.sync.dma_start(out=v0t[:], in_=v0_in)

                st = []
                for g in range(NCH):
                    v_cur = vpool.tile([128, CHUNK], F32R, tag=f"v{g}")
                    nc.vector.tensor_copy(v_cur[:],
                                          v0t[:, 0:1].to_broadcast((128, CHUNK)))
                    st.append({"v": v_cur, "t4": None})

                def dup2_ap(ap, step0, extra_off=0):
                    """AP over [128, 2, CHUNK] with outer step `step0`
                    (0 = duplicate) at offset extra_off."""
                    d = list(ap.ap)
                    return bass.AP(tensor=ap.tensor,
                                   offset=ap.offset + extra_off,
                                   ap=[d[0], [step0, 2], [1, CHUNK]])

                for j in range(K):
                    last = j == K - 1
                    T = [{} for _ in range(NCH)]
                    # ---- gpsimd: fused x*v / y*v front products ----
                    for g in range(NCH):
                        D = T[g]
                        gsl = slice(g * CHUNK, (g + 1) * CHUNK)
                        D["gsl"] = gsl
                        t12 = fr2.tile([128, 2 * CHUNK], F32R, tag=f"t12{g}")
                        xb = 2 * g * CHUNK
                        nc.gpsimd.tensor_mul(t12[:, 0:CHUNK],
                                             xyt[:, xb:xb + CHUNK],
                                             st[g]["v"][:])
                        nc.gpsimd.tensor_mul(t12[:, CHUNK:2 * CHUNK],
                                             xyt[:, xb + CHUNK:xb + 2 * CHUNK],
                                             st[g]["v"][:])
                        D["t12"] = t12
                        D["t4"] = st[g]["t4"]  # prefetched last step
                    # ---- p3/p4 matmuls (PE), then m1/m2 (DVE) ----
                    for g in range(NCH):
                        D = T[g]
                        v_cur = st[g]["v"]
                        D["p3"], D["p4"] = [], []
                        for n0 in range(0, CHUNK, 512):
                            ns = slice(n0, n0 + 512)
                            p3 = pshp.tile([128, 512], F32, tag="psh")
                            nc.tensor.matmul(p3[:], lbt_r[:], v_cur[:, ns],
                                             start=True, stop=True)
                            D["p3"].append(p3)
                            p4 = pshp.tile([128, 512], F32, tag="psh")
                            nc.tensor.matmul(p4[:], lct_r[:], v_cur[:, ns],
                                             start=True, stop=True)
                            D["p4"].append(p4)
                    for g in range(NCH):
                        D = T[g]
                        gsl0 = g * CHUNK
                        m1 = wk1.tile([128, CHUNK], F32R, tag=f"m1{g}")
                        m2 = wk1.tile([128, CHUNK], F32R, tag=f"m2{g}")
                        xb = 2 * g * CHUNK
                        for i, n0 in enumerate(range(0, CHUNK, 512)):
                            ns = slice(n0, n0 + 512)
                            nc.vector.tensor_mul(m1[:, ns],
                                                 xyt[:, xb + n0:xb + n0 + 512],
                                                 D["p3"][i][:])
                            nc.vector.tensor_mul(m2[:, ns],
                                                 xyt[:, xb + CHUNK + n0:
                                                      xb + CHUNK + n0 + 512],
                                                 D["p4"][i][:])
                        D["m1"], D["m2"] = m1, m2
                    # ---- w accumulation in PSUM (PE does all the adds) ----
                    for g in range(NCH):
                        D = T[g]
                        v_cur = st[g]["v"]
                        pw = pwp.tile([128, CHUNK], F32, tag="pw")
                        D["pw"] = pw
                        for n0 in range(0, CHUNK, 512):
                            ns = slice(n0, n0 + 512)
                            nc.tensor.matmul(pw[:, ns], lms_r[:], v_cur[:, ns],
                                             start=True, stop=False)
                            nc.tensor.matmul(pw[:, ns], lbf_r[:],
                                             D["t12"][:, n0:n0 + 512],
                                             start=False, stop=False)
                            nc.tensor.matmul(pw[:, ns], identR[:], D["m1"][:, ns],
                                             start=False, stop=False)
                            nc.tensor.matmul(pw[:, ns], identR[:], D["m2"][:, ns],
                                             start=False, stop=False)
                            if j > 0:
                                nc.tensor.matmul(pw[:, ns], identN[:],
                                                 D["t4"][:, ns],
                                                 start=False, stop=False)
                            nc.tensor.matmul(pw[:, ns], lcf_r[:],
                                             D["t12"][:, CHUNK + n0:CHUNK + n0 + 512],
                                             start=False, stop=True)
                    # ---- alpha: p = v*w (DVE), obd matmul, stage via as_ ----
                    for g in range(NCH):
                        D = T[g]
                        p_t = fr1.tile([128, CHUNK], F32R, tag=f"pt{g}")
                        for n0 in range(0, CHUNK, 512):
                            ns = slice(n0, n0 + 512)
                            nc.vector.tensor_mul(p_t[:, ns], st[g]["v"][:, ns],
                                                 D["pw"][:, ns])
                        D["p_t"] = p_t
                    for g in range(NCH):
                        D = T[g]
                        D["ab"] = []
                        for n0 in range(0, CHUNK, 512):
                            ab = pshp.tile([128, 512], F32, tag="psh")
                            D["ab"].append(ab)
                            nc.tensor.matmul(ab[:], obd_r[:],
                                             D["p_t"][:, n0:n0 + 512],
                                             start=True, stop=True)
                    if not last:
                        # mav = alpha*v straight from PSUM; w~ = pw - mav
                        for g in range(NCH):
                            D = T[g]
                            mav = wk1.tile([128, CHUNK], F32, tag=f"mav{g}")
                            for i, n0 in enumerate(range(0, CHUNK, 512)):
                                ns = slice(n0, n0 + 512)
                                nc.vector.tensor_mul(mav[:, ns], D["ab"][i][:],
                                                     st[g]["v"][:, ns])
                            wt = wtp.tile([128, CHUNK], F32, tag=f"wt{g}")
                            nc.vector.tensor_sub(wt[:], D["pw"][:], mav[:])
                            D["wt"] = wt
                    # alpha broadcast copy (off critical path, staging only)
                    for g in range(NCH):
                        D = T[g]
                        as_ = wk2.tile([128, CHUNK], F32, tag=f"as{g}")
                        for i, n0 in enumerate(range(0, CHUNK, 512)):
                            nc.scalar.activation(as_[:, n0:n0 + 512],
                                                 D["ab"][i][:], ACTF.Copy)
                        gsl = D["gsl"]
                        nc.sync.dma_start(out=ta_tall[2 * j:2 * j + 1, gsl],
                                          in_=as_[0:1, :])
                        nc.sync.dma_start(out=ta_tall[2 * j + 1:2 * j + 2, gsl],
                                          in_=as_[64:65, :])
                    if last:
                        continue
                    # ---- beta: q = wt^2 on Scalar, obd, sqrt, stage ----
                    for g in range(NCH):
                        D = T[g]
                        q_t = fr1.tile([128, CHUNK], F32R, tag=f"qt{g}")
                        for n0 in range(0, CHUNK, 512):
                            ns = slice(n0, n0 + 512)
                            nc.scalar.activation(q_t[:, ns], D["wt"][:, ns],
                                                 ACTF.Square)
                        D["q_t"] = q_t
                    for g in range(NCH):
                        D = T[g]
                        D["b2"] = []
                        for n0 in range(0, CHUNK, 512):
                            b2 = pshp.tile([128, 512], F32, tag="psh")
                            D["b2"].append(b2)
                            nc.tensor.matmul(b2[:], obd_r[:],
                                             D["q_t"][:, n0:n0 + 512],
                                             start=True, stop=True)
                    for g in range(NCH):
                        D = T[g]
                        bb = wk2.tile([128, CHUNK], F32, tag=f"bb{g}")
                        for i, n0 in enumerate(range(0, CHUNK, 512)):
                            nc.scalar.activation(bb[:, n0:n0 + 512],
                                                 D["b2"][i][:], ACTF.Sqrt,
                                                 bias=epst[:], scale=1.0)
                        D["bb"] = bb
                        gsl = D["gsl"]
                        nc.sync.dma_start(out=bb_tall[2 * j:2 * j + 1, gsl],
                                          in_=bb[0:1, :])
                        nc.sync.dma_start(out=bb_tall[2 * j + 1:2 * j + 2, gsl],
                                          in_=bb[64:65, :])
                    # ---- prefetch next step's recurrence tail on gpsimd ----
                    for g in range(NCH):
                        D = T[g]
                        t4n = fr1.tile([128, CHUNK], F32R, tag=f"t4{g}")
                        nc.gpsimd.tensor_mul(t4n[:], D["bb"][:], st[g]["v"][:])
                        st[g]["t4"] = t4n
                    # ---- normalize: v_next = wt * (1/bb) ----
                    for g in range(NCH):
                        D = T[g]
                        rb = wk1.tile([128, CHUNK], F32, tag=f"rb{g}")
                        nc.vector.reciprocal_approx_fast(out=rb[:], in_=D["bb"][:])
                        v_nxt = vpool.tile([128, CHUNK], F32R, tag=f"v{g}")
                        nc.vector.tensor_mul(v_nxt[:], D["wt"][:], rb[:])
                        st[g]["v"] = v_nxt

            # ---------------- Phase 2: transpose + Sturm bisection ----------
            with (
                tc.tile_pool(name="bis", bufs=1) as bis,
                tc.tile_pool(name="cap", bufs=3) as cap,
                tc.tile_pool(name="sl2", bufs=4) as sl2,
                tc.tile_pool(name="pt", bufs=2, space="PSUM") as ptp,
            ):
                ident = bis.tile([128, 128], F32)
                make_identity(nc, ident[:])

                td_a = bis.tile([128, TG, ROWS_A], F32)
                absb = bis.tile([128, TG, ROWS_B], F32)
                for t in range(TG):
                    csl = slice(t * 128, (t + 1) * 128)
                    pa = ptp.tile([128, ROWS_A], F32, tag="pt")
                    nc.tensor.transpose(pa[:], ta_tall[:, csl],
                                        ident[0:ROWS_A, 0:ROWS_A])
                    nc.vector.tensor_copy(td_a[:, t, :], pa[:])
                    pb = ptp.tile([128, ROWS_B], F32, tag="pt")
                    nc.tensor.transpose(pb[:], bb_tall[:, csl],
                                        ident[0:ROWS_B, 0:ROWS_B])
                    nc.vector.tensor_copy(absb[:, t, :], pb[:])

                import concourse.bass as bass_mod

                def jdims_ap(tile_ap, nj, step0=2):
                    d = list(tile_ap.ap)
                    return bass_mod.AP(
                        tensor=tile_ap.tensor, offset=tile_ap.offset,
                        ap=[d[0], d[1], [1, 2], [step0, nj]],
                    )

                td_b = bis.tile([128, TG, ROWS_B], F32)
                nc.vector.tensor_mul(td_b[:], absb[:], absb[:])

                g = bis.tile([128, TG, ROWS_A], F32)
                nc.vector.tensor_copy(g[:], td_a[:])
                nc.vector.tensor_sub(g[:, :, 2:ROWS_A], g[:, :, 2:ROWS_A], absb[:])
                nc.vector.tensor_sub(g[:, :, 0:ROWS_B], g[:, :, 0:ROWS_B], absb[:])

                lo = bis.tile([128, TG, 2], F32)
                hi = bis.tile([128, TG, 2], F32)
                nc.vector.tensor_reduce(lo[:], jdims_ap(g[:], K),
                                        mybir.AxisListType.X, OP.min)
                if idx == 0:
                    nc.vector.tensor_reduce(hi[:], jdims_ap(td_a[:], K),
                                            mybir.AxisListType.X, OP.min)
                else:
                    g2 = g
                    nc.vector.tensor_copy(g2[:], td_a[:])
                    nc.vector.tensor_add(g2[:, :, 2:ROWS_A], g2[:, :, 2:ROWS_A],
                                         absb[:])
                    nc.vector.tensor_add(g2[:, :, 0:ROWS_B], g2[:, :, 0:ROWS_B],
                                         absb[:])
                    nc.vector.tensor_reduce(hi[:], jdims_ap(g2[:], K),
                                            mybir.AxisListType.X, OP.max)

                cs = bis.tile([128, NS, TG, 2], F32)
                for s in range(NS):
                    nc.vector.memset(cs[:, s, :, :], float(s + 1) / float(NS + 1))

                sig = bis.tile([128, NS, TG, 2], F32)
                d_t = bis.tile([128, TG, 2], F32)
                EV, EP = nc.vector, nc.gpsimd
                halves = [
                    (EV, slice(0, TSPLIT), TSPLIT, "d"),
                    (EP, slice(TSPLIT, TG), TG - TSPLIT, "p"),
                ]
                rec = {}
                for _, _, tn, nm in halves:
                    rec[nm] = {
                        k: bis.tile([128, NS, tn, 2], F32, name=f"{k}{nm}")
                        for k in ("pA", "pB", "pC", "cA", "cB")
                    }
                mle = bis.tile([128, TG, 2], I32)
                mgt = bis.tile([128, TG, 2], I32)

                def tg_ap(tile_ap, tsl, extra_off=0, count=2):
                    d = list(tile_ap.ap)
                    part = d[0]
                    tdim = list(d[1])
                    tn = tsl.stop - tsl.start
                    return bass_mod.AP(
                        tensor=tile_ap.tensor,
                        offset=tile_ap.offset + tsl.start * tdim[0] + extra_off,
                        ap=[part, [0, NS], [tdim[0], tn], [1, count]],
                    )

                thr = float(idx) + 0.5
                for ip in range(PASSES):
                    nc.vector.tensor_sub(d_t[:], hi[:], lo[:])
                    nc.vector.tensor_mul(sig[:], cs[:], _bcast_flat(d_t[:]))
                    nc.vector.tensor_add(sig[:], sig[:], _bcast_flat(lo[:]))
                    for eng, tsl, tn, nm in halves:
                        R = rec[nm]
                        po, pp, pn = R["pA"], R["pB"], R["pC"]
                        cnt, cnt_nxt = R["cA"], R["cB"]
                        eng.memset(po[:], 1.0)
                        eng.tensor_sub(pp[:], tg_ap(td_a[:], tsl, 0),
                                       sig[:, :, tsl, :])
                        nc.vector.tensor_scalar(out=cnt[:], in0=pp[:],
                                                scalar1=0.0, scalar2=None,
                                                op0=OP.is_lt)
                        R["rot"] = (po, pp, pn, cnt, cnt_nxt)
                    for j in range(1, K):
                        for eng, tsl, tn, nm in halves:
                            R = rec[nm]
                            po, pp, pn, cnt, cnt_nxt = R["rot"]
                            ca_t = cap.tile([128, NS, tn, 2], F32, tag=f"ca{nm}")
                            eng.tensor_sub(ca_t[:], tg_ap(td_a[:], tsl, 2 * j),
                                           sig[:, :, tsl, :])
                            u_t = sl2.tile([128, NS, tn, 2], F32, tag=f"u{nm}")
                            eng.tensor_mul(u_t[:], ca_t[:], pp[:])
                            tb_t = sl2.tile([128, NS, tn, 2], F32, tag=f"tb{nm}")
                            eng.tensor_mul(tb_t[:],
                                           tg_ap(td_b[:], tsl, 2 * (j - 1)),
                                           po[:])
                            eng.tensor_sub(pn[:], u_t[:], tb_t[:])
                            sg_t = sl2.tile([128, NS, tn, 2], F32, tag=f"sg{nm}")
                            eng.tensor_mul(sg_t[:], pn[:], pp[:])
                            nc.vector.scalar_tensor_tensor(
                                out=cnt_nxt[:], in0=sg_t[:], scalar=0.0,
                                in1=cnt[:], op0=OP.is_lt, op1=OP.add)
                            R["rot"] = (pp, pn, po, cnt_nxt, cnt)
                    for s in range(NS):
                        for eng2, tsl, tn, nm in halves:
                            cnt = rec[nm]["rot"][3]
                            nc.vector.tensor_scalar(
                                out=mle[:, tsl, :], in0=cnt[:, s, :, :],
                                scalar1=thr, scalar2=None, op0=OP.is_le)
                        nc.vector.copy_predicated(out=lo[:], mask=mle[:],
                                                  data=sig[:, s, :, :])
                    for s in range(NS - 1, -1, -1):
                        for eng2, tsl, tn, nm in halves:
                            cnt = rec[nm]["rot"][3]
                            nc.vector.tensor_scalar(
                                out=mgt[:, tsl, :], in0=cnt[:, s, :, :],
                                scalar1=thr, scalar2=None, op0=OP.is_gt)
                        nc.vector.copy_predicated(out=hi[:], mask=mgt[:],
                                                  data=sig[:, s, :, :])

                lam_t = bis.tile([128, TG, 2], F32)
                nc.vector.tensor_add(lam_t[:], lo[:], hi[:])
                nc.vector.tensor_scalar(out=lam_t[:], in0=lam_t[:],
                                        scalar1=OUT_SCALE, scalar2=None,
                                        op0=OP.mult)
                lam_ap = lam_out.rearrange("(h t p) -> h p t", h=2, t=TG, p=128)
                for h in range(2):
                    nc.sync.dma_start(out=lam_ap[h], in_=lam_t[:, :, h])

    nc.compile()
    return nc


def kernel(x, y, A, B, C, eigval_idx):
    from concourse.bass_utils import run_bass_kernel_spmd

    idx = int(np.asarray(eigval_idx))
    nc = _program(idx)

    A32 = np.asarray(A, np.float32) * C_OP
    B32 = np.asarray(B, np.float32) * C_OP
    C32 = np.asarray(C, np.float32) * C_OP
    lms = _bd(A32 + A32.T)
    lbf = _bd(B32.T)
    lcf = _bd(C32.T)
    lbt = _bd(B32)
    lct = _bd(C32)
    obd = _bd(np.ones((64, 64), np.float32))
    v0 = np.concatenate([_v0_vec(), _v0_vec()]).reshape(128, 1)

    xT = np.ascontiguousarray(np.asarray(x, np.float32).T)  # (64, BATCH)
    yT = np.ascontiguousarray(np.asarray(y, np.float32).T)

    in_maps = []
    for c in range(NCORES):
        b0 = c * SHARD
        xc = np.concatenate(
            [xT[:, b0: b0 + NFREE], xT[:, b0 + NFREE: b0 + SHARD]], axis=0
        )
        yc = np.concatenate(
            [yT[:, b0: b0 + NFREE], yT[:, b0 + NFREE: b0 + SHARD]], axis=0
        )
        xy = np.empty((128, 2 * NFREE), np.float32)
        for gi in range(NCH):
            g0 = gi * CHUNK
            xy[:, 2 * g0:2 * g0 + CHUNK] = xc[:, g0:g0 + CHUNK]
            xy[:, 2 * g0 + CHUNK:2 * g0 + 2 * CHUNK] = yc[:, g0:g0 + CHUNK]
        in_maps.append(
            {
                "xy": xy,
                "lms": lms, "lbf": lbf, "lcf": lcf, "lbt": lbt, "lct": lct,
                "obd": obd, "v0": v0,
            }
        )

    res = run_bass_kernel_spmd(nc, in_maps, core_ids=list(range(NCORES)))
    out = np.concatenate([res.results[c]["lam"] for c in range(NCORES)])
    return out.reshape(BATCH, 1).astype(np.float32)
